# revision 13
# baseline (speedup 1.0000x reference)
"""Trainium2 Bass kernel for nn_DiscreteTimeNeuralGraph.

Strategy (8 NeuronCores, batch-parallel):
  - Shard the batch of 32 across 8 cores (4 samples each). All weights
    replicated via one mega-weight DMA.
  - Downsample path on-device; BatchNorm batch stats via per-core partial
    sums + one tiny AllReduce each (BN biases cancel through the BNs and
    are dropped).
  - 8 graph layers: depthwise 3x3 conv as 9 rect-clipped diagonal matmuls
    on PE accumulating in PSUM; channel mix (pruned 512x512 weight, dense)
    as blocked matmuls; instance-norm stats on VectorE (bn_stats on PSUM);
    instnorm+ReLU fused into one ScalarE activation reading PSUM and
    writing the next layer's activations.
  - Matmul inputs are float32r (~tf32 precision, 1 col/cycle on PE).
  - Readout: center 2x2 mean (folded into fc weights) + fc matmul.

Top-k threshold for the pruned graph weight is computed on host
(np.partition) -- it is weight preprocessing of a replicated input.
"""
import numpy as np

import concourse.bass as bass
import concourse.tile as tile
from concourse import bacc, mybir
from concourse.bass_utils import run_bass_kernel_spmd

F32 = mybir.dt.float32
F32R = mybir.dt.float32r
AF = mybir.ActivationFunctionType

N_CORES = 8
B = 32
BPC = B // N_CORES          # 4 samples per core
DIM = 512
DS = 128
FEAT = 256
LAYERS = 8
IMG = 128
OUT = 1000
EPS = 1e-5
HALF = IMG // 4 // 2 - 1    # 15
PRUNE = 0.9

# mega-weight column layout (f32r, [128, WCOLS])
W1X_OFF = 0                  # 3 dx-taps x [128,128] for conv1
W2D_OFF = W1X_OFF + 3 * 128  # 9 taps x [128,128] diag-dup for conv2
W3_OFF = W2D_OFF + 9 * 128   # [128,128] conv3 (w3 stacked twice on K)
WDW_OFF = W3_OFF + 128       # 4 groups x 9 taps x [128,128] diag for main dw
WMIX_OFF = WDW_OFF + 36 * 128  # 4 kg x [128, 512] = w_eff.T blocks
FCW_OFF = WMIX_OFF + 4 * 512   # 2 kg x [128, 1000] fc lhsT (x0.25 pooled)
FCB_OFF = FCW_OFF + 2 * 1000   # [128, 8] fc bias chunks
BN1_OFF = FCB_OFF + 8          # [128, 2] bn1 gamma/beta (dup across halves)
BN2_OFF = BN1_OFF + 2          # [128, 2]
NGB_OFF = BN2_OFF + 2          # [128, 8] instnorm gamma/beta per group
Z_OFF = NGB_OFF + 8            # [128, 256] zeros (pad fills)
WCOLS = Z_OFF + 256

# tap order: full-coverage tap first (start=True zeroes the psum region)
TAPS = [(1, 1), (0, 0), (0, 1), (0, 2), (1, 0), (1, 2), (2, 0), (2, 1), (2, 2)]


def _clip(lo, hi, lo2, hi2):
    return max(lo, lo2), min(hi, hi2)


def build_nc():
    nc = bacc.Bacc(num_devices=N_CORES)
    x4 = nc.dram_tensor("x4", [BPC, 3, IMG, IMG], F32R, kind="ExternalInput").ap()
    wts = nc.dram_tensor("wts", [128, WCOLS], F32R, kind="ExternalInput").ap()
    y4 = nc.dram_tensor("y4", [BPC, OUT], F32, kind="ExternalOutput").ap()

    with tile.TileContext(nc) as tc:
        with (
            tc.tile_pool(name="wp", bufs=1) as wp,
            tc.tile_pool(name="small", bufs=1) as small,
            tc.tile_pool(name="psA", bufs=2, space="PSUM") as psA,
            tc.tile_pool(name="psB", bufs=2, space="PSUM") as psB,
            tc.tile_pool(name="dram", bufs=1, space="DRAM") as dram,
        ):
            w_sb = wp.tile([128, WCOLS], F32R)
            nc.sync.dma_start(out=w_sb, in_=wts)
            w32 = w_sb.bitcast(F32)

            def wcols(off, n):
                return w_sb[:, off:off + n]

            def zsrc(d1, d2):
                return bass.AP(tensor=w_sb.tensor, offset=w_sb.offset + Z_OFF,
                               ap=[w_sb.ap[0], [d2, d1], [1, d2]])

            eps_t = small.tile([128, 1], F32)
            nc.vector.memset(eps_t, EPS)

            # ---------------- downsample ----------------
            with tc.tile_pool(name="ds1", bufs=1) as ds1:
                # im2col9: partition p = 32*s + 3*dy + c ; free = (oy 64, ix' 130)
                # ix' = ix + 1 (x padded by 1 on both sides)
                im9 = ds1.tile([128, 64 * 130], F32R)
                im9r = im9.rearrange("p (y x) -> p y x", y=64, x=130)
                # zero the x pads (cols 0 and 129) in one DMA
                for xc in (0, 129):
                    im9_pads = bass.AP(tensor=im9.tensor,
                                       offset=im9.offset + xc,
                                       ap=[im9.ap[0], [130, 64]])
                    nc.sync.dma_start(out=im9_pads, in_=wcols(Z_OFF, 64))
                # x rows: iy = 2*oy + dy - 1
                # partition base: sample s -> 64*(s%2) + 27*(s//2)
                x4r = x4.rearrange("s c (y2 two) x -> s c y2 two x", two=2)
                for s in range(BPC):
                    for dy in range(3):
                        p0 = 64 * (s % 2) + 27 * (s // 2) + 3 * dy
                        if dy == 0:
                            # oy in [1,64): iy = 2*(oy-1)+1
                            nc.sync.dma_start(
                                out=im9r[p0:p0 + 3, 1:64, 1:129],
                                in_=x4r[s, :, 0:63, 1, :])
                            # row oy=0 is out of range: zero it
                            nc.sync.dma_start(out=im9r[p0:p0 + 3, 0:1, :],
                                              in_=wcols(Z_OFF, 130)[p0:p0 + 3])
                        elif dy == 1:
                            nc.sync.dma_start(
                                out=im9r[p0:p0 + 3, :, 1:129],
                                in_=x4r[s, :, :, 0, :])
                        else:
                            nc.sync.dma_start(
                                out=im9r[p0:p0 + 3, :, 1:129],
                                in_=x4r[s, :, :, 1, :])

                # conv1: out h1 [128 = 64*(s//2)+ch, (s%2, oy 64, ox 64)]
                h1 = ds1.tile([128, 8192], F32)
                h1r = h1.rearrange("p (sh y x) -> p sh y x", sh=2, y=64, x=64)
                # im2col x-read: ix' = 2*ox + dx (x2 = ox + dx//2, tx = dx%2)
                # paired matmul: K=54 block-diag covers samples (q, q+2):
                # out partitions 0-63 <- sample q, 64-127 <- sample q+2.
                im9x = im9.rearrange("p (y x2 two) -> p y x2 two", x2=65, two=2)
                for q in range(2):
                    for yb in range(4):           # 16-oy blocks
                        pc1 = psA.tile([128, 1024], F32, tag="a", name="pc1")
                        pc1r = pc1.rearrange("p (h y x) -> p h y x",
                                             h=2, y=8, x=64)
                        for h in range(2):
                            oy0 = yb * 16 + h * 8
                            for di, dx in enumerate([1, 0, 2]):
                                rhs = im9x[64 * q:64 * q + 54, oy0:oy0 + 8,
                                           dx // 2:dx // 2 + 64, dx % 2]
                                outp = pc1r[:, h, :, :]
                                lhsT = w_sb[64 * q:64 * q + 54,
                                            W1X_OFF + di_col(dx) * 128:
                                            W1X_OFF + di_col(dx) * 128 + 128]
                                nc.tensor.matmul(outp, lhsT, rhs,
                                                 start=(di == 0), stop=(di == 2),
                                                 tile_position=(64 * q, 0))
                        eng = nc.scalar if (q + yb) % 2 == 0 else nc.vector
                        if eng is nc.scalar:
                            nc.scalar.copy(
                                out=h1r[:, q, yb * 16:yb * 16 + 16, :],
                                in_=pc1)
                        else:
                            nc.vector.tensor_copy(
                                out=h1r[:, q, yb * 16:yb * 16 + 16, :],
                                in_=pc1)

                # BN1 partial stats
                st1 = small.tile([128, 16, 6], F32)
                for i in range(16):
                    nc.vector.bn_stats(out=st1[:, i, :],
                                       in_=h1[:, i * 512:(i + 1) * 512])
                mv1 = small.tile([128, 2], F32)
                nc.vector.bn_aggr(out=mv1, in_=st1)
                sums1 = small.tile([128, 2], F32)
                tmp1 = small.tile([128, 1], F32)
                nc.vector.tensor_scalar_mul(out=sums1[:, 0:1], in0=mv1[:, 0:1],
                                            scalar1=8192.0)
                nc.vector.tensor_mul(out=tmp1, in0=mv1[:, 0:1], in1=mv1[:, 0:1])
                nc.vector.tensor_add(out=tmp1, in0=tmp1, in1=mv1[:, 1:2])
                nc.vector.tensor_scalar_mul(out=sums1[:, 1:2], in0=tmp1,
                                            scalar1=8192.0)
                bn1_in = dram.tile([128, 2], F32)
                bn1_out = dram.tile([128, 2], F32)
                nc.gpsimd.dma_start(out=bn1_in, in_=sums1)
                nc.gpsimd.collective_compute(
                    "AllReduce", mybir.AluOpType.add,
                    replica_groups=[list(range(N_CORES))],
                    ins=[bn1_in.opt()], outs=[bn1_out.opt()])
                red1 = small.tile([128, 2], F32)
                nc.gpsimd.dma_start(out=red1, in_=bn1_out)
                comb1 = small.tile([128, 2], F32)
                nc.gpsimd.dma_start(out=comb1[0:64, :], in_=red1[0:64, :])
                nc.gpsimd.dma_start(out=comb1[0:64, :], in_=red1[64:128, :],
                                    accum_op=mybir.AluOpType.add)
                # scale/bias on rows 0:64, then duplicate
                s1t1 = small.tile([128, 2], F32)
                _bn_scale_bias(nc, s1t1, comb1, w32, BN1_OFF, 131072.0,
                               eps_t, small, rows=64)
                nc.gpsimd.dma_start(out=s1t1[64:128, :], in_=s1t1[0:64, :])

                # apply BN1 + relu -> h1n (f32r), x padded to 66 (ix' = ix+1)
                h1n = ds1.tile([128, 2 * 64 * 66], F32R)
                h1nr3 = h1n.rearrange("p (sh y x) -> p sh y x",
                                      sh=2, y=64, x=66)
                for sh in range(2):
                    for xc in (0, 65):
                        h1n_pads = bass.AP(tensor=h1n.tensor,
                                           offset=h1n.offset + 4224 * sh + xc,
                                           ap=[h1n.ap[0], [66, 64]])
                        nc.sync.dma_start(out=h1n_pads, in_=wcols(Z_OFF, 64))
                h1r4 = h1.rearrange("p (sh y x) -> p sh y x", sh=2, y=64, x=64)
                for sh in range(2):
                    nc.scalar.activation(out=h1nr3[:, sh, :, 1:65],
                                         in_=h1r4[:, sh, :, :], func=AF.Relu,
                                         scale=s1t1[:, 0:1], bias=s1t1[:, 1:2])

                # conv2: depthwise 3x3 stride 2 -> d2 [128, (sh, 32, 32)]
                # row iy = 2*oy + dy - 1 (unpadded), col ix' = 2*ox + dx (padded)
                h1nr = h1n.rearrange(
                    "p (sh y2 ty x2 tx) -> p sh y2 ty x2 tx",
                    sh=2, y2=32, ty=2, x2=33, tx=2)
                d2 = ds1.tile([128, 2048], F32R)
                for sh in range(2):
                    pd2 = psB.tile([128, 1024], F32, tag="b", name="pd2")
                    pd2r = pd2.rearrange("p (h y x) -> p h y x", h=2, y=16, x=32)
                    for h in range(2):
                        for ti, (dy, dx) in enumerate(TAPS):
                            oy0, oy1 = _clip(h * 16, h * 16 + 16,
                                             1 if dy == 0 else 0, 32)
                            if dy == 1:
                                ys, par = oy0, 0
                            elif dy == 0:
                                ys, par = oy0 - 1, 1
                            else:
                                ys, par = oy0, 1
                            rhs = h1nr[:, sh, ys:ys + (oy1 - oy0), par,
                                       dx // 2:dx // 2 + 32, dx % 2]
                            outp = pd2r[:, h, oy0 - h * 16:oy1 - h * 16, :]
                            t = TAPS.index((dy, dx))
                            nc.tensor.matmul(
                                outp, wcols(W2D_OFF + t * 128, 128), rhs,
                                start=(ti == 0), stop=(ti == len(TAPS) - 1))
                    nc.scalar.copy(out=d2[:, sh * 1024:(sh + 1) * 1024], in_=pd2)

                # conv3: 1x1, 64 -> 128 ; h3 [128=outc, (s, 1024px)]
                h3 = small.tile([128, 4096], F32)
                for a in range(2):
                    for half in range(2):
                        pc3 = psA.tile([128, 1024], F32, tag="a",
                                       name=f"pc3_{a}_{half}")
                        for sub in range(2):
                            nb = half * 2 + sub
                            outp = pc3[:, sub * 512:sub * 512 + 512]
                            nc.tensor.matmul(
                                outp,
                                w_sb[64 * a:64 * a + 64, W3_OFF:W3_OFF + 128],
                                d2[64 * a:64 * a + 64, nb * 512:(nb + 1) * 512],
                                start=True, stop=True)
                            s_full = 2 * a + nb // 2
                            dst = h3[:, s_full * 1024 + (nb % 2) * 512:
                                     s_full * 1024 + (nb % 2) * 512 + 512]
                            if nb % 2 == 0:
                                nc.scalar.copy(out=dst, in_=outp)
                            else:
                                nc.vector.tensor_copy(out=dst, in_=outp)

                # BN2 stats + allreduce
                st2 = small.tile([128, 8, 6], F32)
                for i in range(8):
                    nc.vector.bn_stats(out=st2[:, i, :],
                                       in_=h3[:, i * 512:(i + 1) * 512])
                mv2 = small.tile([128, 2], F32)
                nc.vector.bn_aggr(out=mv2, in_=st2)
                sums2 = small.tile([128, 2], F32)
                tmp2 = small.tile([128, 1], F32)
                nc.vector.tensor_scalar_mul(out=sums2[:, 0:1], in0=mv2[:, 0:1],
                                            scalar1=4096.0)
                nc.vector.tensor_mul(out=tmp2, in0=mv2[:, 0:1], in1=mv2[:, 0:1])
                nc.vector.tensor_add(out=tmp2, in0=tmp2, in1=mv2[:, 1:2])
                nc.vector.tensor_scalar_mul(out=sums2[:, 1:2], in0=tmp2,
                                            scalar1=4096.0)
                bn2_in = dram.tile([128, 2], F32)
                bn2_out = dram.tile([128, 2], F32)
                nc.gpsimd.dma_start(out=bn2_in, in_=sums2)
                nc.gpsimd.collective_compute(
                    "AllReduce", mybir.AluOpType.add,
                    replica_groups=[list(range(N_CORES))],
                    ins=[bn2_in.opt()], outs=[bn2_out.opt()])
                red2 = small.tile([128, 2], F32)
                nc.gpsimd.dma_start(out=red2, in_=bn2_out)
                s2t2 = small.tile([128, 2], F32)
                _bn_scale_bias(nc, s2t2, red2, w32, BN2_OFF, 32768.0,
                               eps_t, small, rows=128)

            # ---------------- main loop ----------------
            with (
                tc.tile_pool(name="xp", bufs=20) as xp,
                tc.tile_pool(name="dp", bufs=8) as dp,
                tc.tile_pool(name="stp", bufs=4) as stp,
            ):
                def new_x_tile(name):
                    xt = xp.tile([128, 32 * 34], F32R, tag="X", name=name)
                    for xc in (0, 33):
                        pads = bass.AP(tensor=xt.tensor, offset=xt.offset + xc,
                                       ap=[xt.ap[0], [34, 32]])
                        nc.sync.dma_start(out=pads, in_=wcols(Z_OFF, 32))
                    return xt

                Xcur = {}
                for s in range(BPC):
                    xt = new_x_tile(f"X1_0_{s}")
                    xtr = xt.rearrange("p (y x) -> p y x", y=32, x=34)
                    h3r = h3.rearrange("p (s y x) -> p s y x", s=4, y=32, x=32)
                    nc.scalar.activation(out=xtr[:, :, 1:33],
                                         in_=h3r[:, s, :, :],
                                         func=AF.Relu,
                                         scale=s2t2[:, 0:1], bias=s2t2[:, 1:2])
                    Xcur[(0, s)] = xt

                pooled_in = small.tile([128, 2, 4, 4], F32)

                for L in range(1, LAYERS + 1):
                    gs_in = sorted({g for (g, _s) in Xcur})
                    mgs = [2, 3] if L == LAYERS else [0, 1, 2, 3]
                    Xnext = {}
                    Dcur = {}
                    for s in range(BPC):
                        # depthwise conv for each live group
                        for g in gs_in:
                            Xr = Xcur[(g, s)].rearrange("p (y x) -> p y x",
                                                        y=32, x=34)
                            pdw = psA.tile([128, 1024], F32, tag="a",
                                           name=f"pdw{L}_{s}_{g}")
                            pdwr = pdw.rearrange("p (h y x) -> p h y x",
                                                 h=2, y=16, x=32)
                            for h in range(2):
                                for ti, (dy, dx) in enumerate(TAPS):
                                    oy0, oy1 = _clip(h * 16, h * 16 + 16,
                                                     max(0, 1 - dy), 33 - dy)
                                    if oy0 >= oy1:
                                        continue
                                    rhs = Xr[:, oy0 + dy - 1:oy1 + dy - 1,
                                             dx:dx + 32]
                                    outp = pdwr[:, h, oy0 - h * 16:oy1 - h * 16,
                                                :]
                                    t = TAPS.index((dy, dx))
                                    nc.tensor.matmul(
                                        outp,
                                        wcols(WDW_OFF + (g * 9 + t) * 128, 128),
                                        rhs,
                                        start=(ti == 0),
                                        stop=(ti == len(TAPS) - 1))
                            dD = dp.tile([128, 1024], F32R, tag="D",
                                         name=f"D{L}_{s}_{g}")
                            nc.scalar.copy(out=dD, in_=pdw)
                            Dcur[g] = dD
                        # channel mix + instnorm per output group
                        for mg in mgs:
                            pm = psB.tile([128, 1024], F32, tag="b",
                                          name=f"pm{L}_{s}_{mg}")
                            for h in range(2):
                                for ki, kg in enumerate(gs_in):
                                    nc.tensor.matmul(
                                        pm[:, h * 512:h * 512 + 512],
                                        wcols(WMIX_OFF + kg * 512 + mg * 128,
                                              128),
                                        Dcur[kg][:, h * 512:h * 512 + 512],
                                        start=(ki == 0),
                                        stop=(ki == len(gs_in) - 1))
                            st = stp.tile([128, 2, 6], F32, tag="st")
                            nc.vector.bn_stats(out=st[:, 0, :],
                                               in_=pm[:, 0:512])
                            nc.vector.bn_stats(out=st[:, 1, :],
                                               in_=pm[:, 512:1024])
                            mv = stp.tile([128, 2], F32, tag="mv")
                            nc.vector.bn_aggr(out=mv, in_=st)
                            sc = stp.tile([128, 1], F32, tag="sc")
                            tt = stp.tile([128, 1], F32, tag="tt")
                            nc.scalar.activation(out=sc, in_=mv[:, 1:2],
                                                 func=AF.Sqrt, bias=eps_t)
                            nc.vector.reciprocal(out=sc, in_=sc)
                            nc.vector.tensor_scalar_mul(
                                out=sc, in0=sc,
                                scalar1=w32[:, NGB_OFF + 2 * mg:
                                            NGB_OFF + 2 * mg + 1])
                            nc.vector.tensor_mul(out=tt, in0=mv[:, 0:1], in1=sc)
                            nc.vector.tensor_scalar(
                                out=tt, in0=tt, scalar1=-1.0,
                                scalar2=w32[:, NGB_OFF + 2 * mg + 1:
                                            NGB_OFF + 2 * mg + 2],
                                op0=mybir.AluOpType.mult,
                                op1=mybir.AluOpType.add)
                            if L < LAYERS:
                                xt = new_x_tile(f"X{L + 1}_{mg}_{s}")
                                Xnext[(mg, s)] = xt
                                xtr = xt.rearrange("p (y x) -> p y x",
                                                   y=32, x=34)
                                pmr2 = pm.rearrange("p (h y x) -> p h y x",
                                                    h=2, y=16, x=32)
                                for h in range(2):
                                    nc.scalar.activation(
                                        out=xtr[:, h * 16:h * 16 + 16, 1:33],
                                        in_=pmr2[:, h, :, :],
                                        func=AF.Relu, scale=sc, bias=tt)
                            else:
                                pmr = pm.rearrange("p (h y x) -> p h y x",
                                                   h=2, y=16, x=32)
                                nc.scalar.activation(
                                    out=pooled_in[:, mg - 2, s, :],
                                    in_=pmr[:, 0, HALF - 1:HALF + 1,
                                            HALF - 1:HALF + 1],
                                    func=AF.Identity, scale=sc, bias=tt)
                    Xcur = Xnext

                # ---------------- readout ----------------
                tadd = small.tile([128, 2, 4], F32)
                tadd2 = small.tile([128, 2, 4], F32)
                pooled = small.tile([128, 2, 4], F32R)
                nc.vector.tensor_add(out=tadd, in0=pooled_in[:, :, :, 0],
                                     in1=pooled_in[:, :, :, 1])
                nc.vector.tensor_add(out=tadd2, in0=pooled_in[:, :, :, 2],
                                     in1=pooled_in[:, :, :, 3])
                nc.vector.tensor_add(out=pooled, in0=tadd, in1=tadd2)
                y_sb = small.tile([128, 4, 8], F32)
                for mo in range(8):
                    mlen = 128 if mo < 7 else OUT - 7 * 128
                    pf = psA.tile([128, 1024], F32, tag="a", name=f"pf{mo}")
                    for kgi in range(2):
                        nc.tensor.matmul(
                            pf[0:mlen, 0:4],
                            w_sb[:, FCW_OFF + kgi * 1000 + mo * 128:
                                 FCW_OFF + kgi * 1000 + mo * 128 + mlen],
                            pooled[:, kgi, :],
                            start=(kgi == 0), stop=(kgi == 1))
                    nc.scalar.activation(
                        out=y_sb[0:mlen, :, mo], in_=pf[0:mlen, 0:4],
                        func=AF.Identity,
                        bias=w32[0:mlen, FCB_OFF + mo:FCB_OFF + mo + 1],
                        scale=1.0)
                for s in range(BPC):
                    dst1 = bass.AP(tensor=y4.tensor, offset=OUT * s,
                                   ap=[[1, 128], [128, 7]])
                    nc.sync.dma_start(out=dst1, in_=y_sb[:, s, 0:7])
                    dst2 = bass.AP(tensor=y4.tensor, offset=OUT * s + 896,
                                   ap=[[1, 104]])
                    nc.sync.dma_start(out=dst2, in_=y_sb[0:104, s, 7])

    nc.finalize()
    return nc


def di_col(dx):
    # column index of conv1 tap dx within w1x block (emission order 1,0,2)
    return {1: 0, 0: 1, 2: 2}[dx]


def _bn_scale_bias(nc, out_st, sums, w32, gb_off, n_tot, eps_t, pool, rows):
    """out_st[:rows, 0] = gamma*rsqrt(var+eps); out_st[:rows, 1] = beta - mu*scale."""
    r = slice(0, rows)
    mu = pool.tile([128, 1], F32, name=f"mu{gb_off}")
    ex2 = pool.tile([128, 1], F32, name=f"ex2{gb_off}")
    var = pool.tile([128, 1], F32, name=f"var{gb_off}")
    nc.vector.tensor_scalar_mul(out=mu[r], in0=sums[r, 0:1], scalar1=1.0 / n_tot)
    nc.vector.tensor_scalar_mul(out=ex2[r], in0=sums[r, 1:2], scalar1=1.0 / n_tot)
    nc.vector.tensor_mul(out=var[r], in0=mu[r], in1=mu[r])
    nc.vector.tensor_sub(out=var[r], in0=ex2[r], in1=var[r])
    nc.scalar.activation(out=var[r], in_=var[r], func=AF.Sqrt, bias=eps_t[r])
    nc.vector.reciprocal(out=var[r], in_=var[r])
    nc.vector.tensor_scalar_mul(out=out_st[r, 0:1], in0=var[r],
                                scalar1=w32[r, gb_off:gb_off + 1])
    nc.vector.tensor_mul(out=mu[r], in0=mu[r], in1=out_st[r, 0:1])
    nc.vector.tensor_scalar(out=out_st[r, 1:2], in0=mu[r], scalar1=-1.0,
                            scalar2=w32[r, gb_off + 1:gb_off + 2],
                            op0=mybir.AluOpType.mult,
                            op1=mybir.AluOpType.add)


def _pack_weights(ds_w1, ds_w2, ds_w3, conv_w, graph_w, fc_w, fc_b,
                  bn1_g, bn1_b, bn2_g, bn2_b, norm_g, norm_b):
    wts = np.zeros((128, WCOLS), np.float32)
    # pruned graph weight
    k = int((1.0 - PRUNE) * DIM * DIM)
    a = np.abs(graph_w).ravel()
    thresh = np.partition(a, -k)[-k]
    w_eff = np.where(np.abs(graph_w) >= thresh, graph_w, 0.0).astype(np.float32)
    # conv1 taps, paired block-diag:
    # rows 64*q + 27*a + 3*dy + c, cols 64*a + o = w1[o, c, dy, dx]
    for dx in range(3):
        dc = di_col(dx)
        blk = np.zeros((128, 128), np.float32)
        for qq in range(2):
            for aa in range(2):
                for dy in range(3):
                    for c in range(3):
                        blk[64 * qq + 27 * aa + 3 * dy + c,
                            64 * aa:64 * aa + 64] = ds_w1[:, c, dy, dx]
        wts[:, W1X_OFF + dc * 128:W1X_OFF + (dc + 1) * 128] = blk
    # conv2 diag-dup taps
    for t, (dy, dx) in enumerate(TAPS):
        blk = np.zeros((128, 128), np.float32)
        d = ds_w2[:, 0, dy, dx]
        for aa in range(2):
            idx = np.arange(64)
            blk[64 * aa + idx, 64 * aa + idx] = d
        wts[:, W2D_OFF + t * 128:W2D_OFF + (t + 1) * 128] = blk
    # conv3: [64a + c, o] = w3[o, c]
    w3 = ds_w3[:, :, 0, 0]  # [128, 64]
    wts[0:64, W3_OFF:W3_OFF + 128] = w3.T
    wts[64:128, W3_OFF:W3_OFF + 128] = w3.T
    # main dw diag taps
    for g in range(4):
        for t, (dy, dx) in enumerate(TAPS):
            blk = np.zeros((128, 128), np.float32)
            idx = np.arange(128)
            blk[idx, idx] = conv_w[g * 128:(g + 1) * 128, 0, dy, dx]
            off = WDW_OFF + (g * 9 + t) * 128
            wts[:, off:off + 128] = blk
    # mix: [p, kg*512 + mg*128 + j] = w_eff[mg*128 + j, kg*128 + p]
    weT = w_eff.T  # [in, out]
    for kg in range(4):
        wts[:, WMIX_OFF + kg * 512:WMIX_OFF + (kg + 1) * 512] = \
            weT[kg * 128:(kg + 1) * 128, :]
    # fc: [p, kg*1000 + m] = 0.25 * fc_w[m, kg*128 + p]
    for kg in range(2):
        wts[:, FCW_OFF + kg * 1000:FCW_OFF + (kg + 1) * 1000] = \
            0.25 * fc_w[:, kg * 128:(kg + 1) * 128].T
    # fc bias [p, mo]
    fcb = np.zeros((128, 8), np.float32)
    fb = np.zeros(1024, np.float32)
    fb[:OUT] = fc_b
    fcb[:, :] = fb.reshape(8, 128).T
    wts[:, FCB_OFF:FCB_OFF + 8] = fcb
    # bn gammas/betas
    wts[0:64, BN1_OFF] = bn1_g
    wts[64:128, BN1_OFF] = bn1_g
    wts[0:64, BN1_OFF + 1] = bn1_b
    wts[64:128, BN1_OFF + 1] = bn1_b
    wts[:, BN2_OFF] = bn2_g
    wts[:, BN2_OFF + 1] = bn2_b
    for g in range(4):
        wts[:, NGB_OFF + 2 * g] = norm_g[g * 128:(g + 1) * 128]
        wts[:, NGB_OFF + 2 * g + 1] = norm_b[g * 128:(g + 1) * 128]
    return wts


_nc_cache = None
last_results = None


def kernel(**inputs):
    global _nc_cache, last_results
    inputs = {k: np.asarray(v, np.float32) for k, v in inputs.items()}
    wts = _pack_weights(
        inputs["ds_w1"], inputs["ds_w2"], inputs["ds_w3"], inputs["conv_w"],
        inputs["graph_w"], inputs["fc_w"], inputs["fc_b"],
        inputs["bn1_g"], inputs["bn1_b"], inputs["bn2_g"], inputs["bn2_b"],
        inputs["norm_g"], inputs["norm_b"])
    x = inputs["x"]
    if _nc_cache is None:
        _nc_cache = build_nc()
    nc = _nc_cache
    in_maps = [{"x4": np.ascontiguousarray(x[c * BPC:(c + 1) * BPC]),
                "wts": wts} for c in range(N_CORES)]
    res = run_bass_kernel_spmd(nc, in_maps, core_ids=list(range(N_CORES)))
    last_results = res
    return np.concatenate([res.results[c]["y4"] for c in range(N_CORES)], axis=0)


# revision 14
# speedup vs baseline: 1319.5030x; 1319.5030x over previous
"""Trainium2 Bass kernel for nn_DiscreteTimeNeuralGraph.

Strategy (8 NeuronCores, batch-parallel):
  - Shard the batch of 32 across 8 cores (4 samples each). All weights
    replicated via one mega-weight DMA.
  - Downsample path on-device; BatchNorm batch stats via per-core partial
    sums + one tiny AllReduce each (BN biases cancel through the BNs and
    are dropped).
  - 8 graph layers: depthwise 3x3 conv as 9 rect-clipped diagonal matmuls
    on PE accumulating in PSUM; channel mix (pruned 512x512 weight, dense)
    as blocked matmuls; instance-norm stats on VectorE (bn_stats on PSUM);
    instnorm+ReLU fused into one ScalarE activation reading PSUM and
    writing the next layer's activations.
  - Matmul inputs are float32r (~tf32 precision, 1 col/cycle on PE).
  - Readout: center 2x2 mean (folded into fc weights) + fc matmul.

Top-k threshold for the pruned graph weight is computed on host
(np.partition) -- it is weight preprocessing of a replicated input.
"""
import numpy as np

import concourse.bass as bass
import concourse.tile as tile
from concourse import bacc, mybir
from concourse.bass_utils import run_bass_kernel_spmd

F32 = mybir.dt.float32
F32R = mybir.dt.float32r
AF = mybir.ActivationFunctionType

N_CORES = 8
B = 32
BPC = B // N_CORES          # 4 samples per core
DIM = 512
DS = 128
FEAT = 256
LAYERS = 8
IMG = 128
OUT = 1000
EPS = 1e-5
HALF = IMG // 4 // 2 - 1    # 15
PRUNE = 0.9

# mega-weight column layout (f32r, [128, WCOLS])
W1X_OFF = 0                  # 3 dx-taps x [128,128] for conv1
W2D_OFF = W1X_OFF + 3 * 128  # 9 taps x [128,128] diag-dup for conv2
W3_OFF = W2D_OFF + 9 * 128   # [128,128] conv3 (w3 stacked twice on K)
WDW_OFF = W3_OFF + 128       # 4 groups x 9 taps x [128,128] diag for main dw
WMIX_OFF = WDW_OFF + 36 * 128  # 4 kg x [128, 512] = w_eff.T blocks
FCW_OFF = WMIX_OFF + 4 * 512   # 2 kg x [128, 1000] fc lhsT (x0.25 pooled)
FCB_OFF = FCW_OFF + 2 * 1000   # [128, 8] fc bias chunks
BN1_OFF = FCB_OFF + 8          # [128, 2] bn1 gamma/beta (dup across halves)
BN2_OFF = BN1_OFF + 2          # [128, 2]
NGB_OFF = BN2_OFF + 2          # [128, 8] instnorm gamma/beta per group
Z_OFF = NGB_OFF + 8            # [128, 256] zeros (pad fills)
WCOLS = Z_OFF + 256

XP_BUFS = 20

# tap order: full-coverage tap first (start=True zeroes the psum region)
TAPS = [(1, 1), (0, 0), (0, 1), (0, 2), (1, 0), (1, 2), (2, 0), (2, 1), (2, 2)]


def _clip(lo, hi, lo2, hi2):
    return max(lo, lo2), min(hi, hi2)


def build_nc():
    nc = bacc.Bacc(num_devices=N_CORES)
    x4 = nc.dram_tensor("x4", [BPC, 3, IMG, IMG], F32R, kind="ExternalInput").ap()
    wts = nc.dram_tensor("wts", [128, WCOLS], F32R, kind="ExternalInput").ap()
    y4 = nc.dram_tensor("y4", [BPC, OUT], F32, kind="ExternalOutput").ap()

    with tile.TileContext(nc) as tc:
        with (
            tc.tile_pool(name="wp", bufs=1) as wp,
            tc.tile_pool(name="small", bufs=1) as small,
            tc.tile_pool(name="psA", bufs=2, space="PSUM") as psA,
            tc.tile_pool(name="psB", bufs=2, space="PSUM") as psB,
            tc.tile_pool(name="dram", bufs=1, space="DRAM") as dram,
        ):
            w_sb = wp.tile([128, WCOLS], F32R)
            nc.sync.dma_start(out=w_sb, in_=wts)
            w32 = w_sb.bitcast(F32)

            def wcols(off, n):
                return w_sb[:, off:off + n]

            def zsrc(d1, d2):
                return bass.AP(tensor=w_sb.tensor, offset=w_sb.offset + Z_OFF,
                               ap=[w_sb.ap[0], [d2, d1], [1, d2]])

            eps_t = small.tile([128, 1], F32)
            nc.vector.memset(eps_t, EPS)

            # ---------------- downsample ----------------
            with tc.tile_pool(name="ds1", bufs=1) as ds1:
                # im2col9: partition p = 32*s + 3*dy + c ; free = (oy 64, ix' 130)
                # ix' = ix + 1 (x padded by 1 on both sides)
                im9 = ds1.tile([128, 64 * 130], F32R)
                im9r = im9.rearrange("p (y x) -> p y x", y=64, x=130)
                # zero the x pads (cols 0 and 129) in one DMA
                for xc in (0, 129):
                    im9_pads = bass.AP(tensor=im9.tensor,
                                       offset=im9.offset + xc,
                                       ap=[im9.ap[0], [130, 64]])
                    nc.sync.dma_start(out=im9_pads, in_=wcols(Z_OFF, 64))
                # x rows: iy = 2*oy + dy - 1
                # partition base: sample s -> 64*(s%2) + 27*(s//2)
                x4r = x4.rearrange("s c (y2 two) x -> s c y2 two x", two=2)
                for s in range(BPC):
                    for dy in range(3):
                        p0 = 64 * (s % 2) + 27 * (s // 2) + 3 * dy
                        if dy == 0:
                            # oy in [1,64): iy = 2*(oy-1)+1
                            nc.sync.dma_start(
                                out=im9r[p0:p0 + 3, 1:64, 1:129],
                                in_=x4r[s, :, 0:63, 1, :])
                            # row oy=0 is out of range: zero it
                            nc.sync.dma_start(out=im9r[p0:p0 + 3, 0:1, :],
                                              in_=wcols(Z_OFF, 130)[p0:p0 + 3])
                        elif dy == 1:
                            nc.sync.dma_start(
                                out=im9r[p0:p0 + 3, :, 1:129],
                                in_=x4r[s, :, :, 0, :])
                        else:
                            nc.sync.dma_start(
                                out=im9r[p0:p0 + 3, :, 1:129],
                                in_=x4r[s, :, :, 1, :])

                # conv1: out h1 [128 = 64*(s//2)+ch, (s%2, oy 64, ox 64)]
                h1 = ds1.tile([128, 8192], F32)
                h1r = h1.rearrange("p (sh y x) -> p sh y x", sh=2, y=64, x=64)
                # im2col x-read: ix' = 2*ox + dx (x2 = ox + dx//2, tx = dx%2)
                # paired matmul: K=54 block-diag covers samples (q, q+2):
                # out partitions 0-63 <- sample q, 64-127 <- sample q+2.
                im9x = im9.rearrange("p (y x2 two) -> p y x2 two", x2=65, two=2)
                for q in range(2):
                    for yb in range(4):           # 16-oy blocks
                        pc1 = psA.tile([128, 1024], F32, tag="a", name="pc1")
                        pc1r = pc1.rearrange("p (h y x) -> p h y x",
                                             h=2, y=8, x=64)
                        for h in range(2):
                            oy0 = yb * 16 + h * 8
                            for di, dx in enumerate([1, 0, 2]):
                                rhs = im9x[64 * q:64 * q + 54, oy0:oy0 + 8,
                                           dx // 2:dx // 2 + 64, dx % 2]
                                outp = pc1r[:, h, :, :]
                                lhsT = w_sb[64 * q:64 * q + 54,
                                            W1X_OFF + di_col(dx) * 128:
                                            W1X_OFF + di_col(dx) * 128 + 128]
                                nc.tensor.matmul(outp, lhsT, rhs,
                                                 start=(di == 0), stop=(di == 2),
                                                 tile_position=(64 * q, 0))
                        eng = nc.scalar if (q + yb) % 2 == 0 else nc.vector
                        if eng is nc.scalar:
                            nc.scalar.copy(
                                out=h1r[:, q, yb * 16:yb * 16 + 16, :],
                                in_=pc1)
                        else:
                            nc.vector.tensor_copy(
                                out=h1r[:, q, yb * 16:yb * 16 + 16, :],
                                in_=pc1)

                # BN1 partial stats
                st1 = small.tile([128, 16, 6], F32)
                for i in range(16):
                    nc.vector.bn_stats(out=st1[:, i, :],
                                       in_=h1[:, i * 512:(i + 1) * 512])
                mv1 = small.tile([128, 2], F32)
                nc.vector.bn_aggr(out=mv1, in_=st1)
                sums1 = small.tile([128, 2], F32)
                tmp1 = small.tile([128, 1], F32)
                nc.vector.tensor_scalar_mul(out=sums1[:, 0:1], in0=mv1[:, 0:1],
                                            scalar1=8192.0)
                nc.vector.tensor_mul(out=tmp1, in0=mv1[:, 0:1], in1=mv1[:, 0:1])
                nc.vector.tensor_add(out=tmp1, in0=tmp1, in1=mv1[:, 1:2])
                nc.vector.tensor_scalar_mul(out=sums1[:, 1:2], in0=tmp1,
                                            scalar1=8192.0)
                bn1_in = dram.tile([128, 2], F32)
                bn1_out = dram.tile([128, 2], F32)
                nc.gpsimd.dma_start(out=bn1_in, in_=sums1)
                nc.gpsimd.collective_compute(
                    "AllReduce", mybir.AluOpType.add,
                    replica_groups=[list(range(N_CORES))],
                    ins=[bn1_in.opt()], outs=[bn1_out.opt()])
                red1 = small.tile([128, 2], F32)
                nc.gpsimd.dma_start(out=red1, in_=bn1_out)
                comb1 = small.tile([128, 2], F32)
                nc.gpsimd.dma_start(out=comb1[0:64, :], in_=red1[0:64, :])
                nc.gpsimd.dma_start(out=comb1[0:64, :], in_=red1[64:128, :],
                                    accum_op=mybir.AluOpType.add)
                # scale/bias on rows 0:64, then duplicate
                s1t1 = small.tile([128, 2], F32)
                _bn_scale_bias(nc, s1t1, comb1, w32, BN1_OFF, 131072.0,
                               eps_t, small, rows=64)
                nc.gpsimd.dma_start(out=s1t1[64:128, :], in_=s1t1[0:64, :])

                # apply BN1 + relu -> h1n (f32r), x padded to 66 (ix' = ix+1)
                h1n = ds1.tile([128, 2 * 64 * 66], F32R)
                h1nr3 = h1n.rearrange("p (sh y x) -> p sh y x",
                                      sh=2, y=64, x=66)
                for sh in range(2):
                    for xc in (0, 65):
                        h1n_pads = bass.AP(tensor=h1n.tensor,
                                           offset=h1n.offset + 4224 * sh + xc,
                                           ap=[h1n.ap[0], [66, 64]])
                        nc.sync.dma_start(out=h1n_pads, in_=wcols(Z_OFF, 64))
                h1r4 = h1.rearrange("p (sh y x) -> p sh y x", sh=2, y=64, x=64)
                for sh in range(2):
                    nc.scalar.activation(out=h1nr3[:, sh, :, 1:65],
                                         in_=h1r4[:, sh, :, :], func=AF.Relu,
                                         scale=s1t1[:, 0:1], bias=s1t1[:, 1:2])

                # conv2: depthwise 3x3 stride 2 -> d2 [128, (sh, 32, 32)]
                # row iy = 2*oy + dy - 1 (unpadded), col ix' = 2*ox + dx (padded)
                h1nr = h1n.rearrange(
                    "p (sh y2 ty x2 tx) -> p sh y2 ty x2 tx",
                    sh=2, y2=32, ty=2, x2=33, tx=2)
                d2 = ds1.tile([128, 2048], F32R)
                for sh in range(2):
                    pd2 = psB.tile([128, 1024], F32, tag="b", name="pd2")
                    pd2r = pd2.rearrange("p (h y x) -> p h y x", h=2, y=16, x=32)
                    for h in range(2):
                        for ti, (dy, dx) in enumerate(TAPS):
                            oy0, oy1 = _clip(h * 16, h * 16 + 16,
                                             1 if dy == 0 else 0, 32)
                            if dy == 1:
                                ys, par = oy0, 0
                            elif dy == 0:
                                ys, par = oy0 - 1, 1
                            else:
                                ys, par = oy0, 1
                            rhs = h1nr[:, sh, ys:ys + (oy1 - oy0), par,
                                       dx // 2:dx // 2 + 32, dx % 2]
                            outp = pd2r[:, h, oy0 - h * 16:oy1 - h * 16, :]
                            t = TAPS.index((dy, dx))
                            nc.tensor.matmul(
                                outp, wcols(W2D_OFF + t * 128, 128), rhs,
                                start=(ti == 0), stop=(ti == len(TAPS) - 1))
                    nc.scalar.copy(out=d2[:, sh * 1024:(sh + 1) * 1024], in_=pd2)

                # conv3: 1x1, 64 -> 128 ; h3 [128=outc, (s, 1024px)]
                h3 = small.tile([128, 4096], F32)
                for a in range(2):
                    for half in range(2):
                        pc3 = psA.tile([128, 1024], F32, tag="a",
                                       name=f"pc3_{a}_{half}")
                        for sub in range(2):
                            nb = half * 2 + sub
                            outp = pc3[:, sub * 512:sub * 512 + 512]
                            nc.tensor.matmul(
                                outp,
                                w_sb[64 * a:64 * a + 64, W3_OFF:W3_OFF + 128],
                                d2[64 * a:64 * a + 64, nb * 512:(nb + 1) * 512],
                                start=True, stop=True)
                            s_full = 2 * a + nb // 2
                            dst = h3[:, s_full * 1024 + (nb % 2) * 512:
                                     s_full * 1024 + (nb % 2) * 512 + 512]
                            if nb % 2 == 0:
                                nc.scalar.copy(out=dst, in_=outp)
                            else:
                                nc.vector.tensor_copy(out=dst, in_=outp)

                # BN2 stats + allreduce
                st2 = small.tile([128, 8, 6], F32)
                for i in range(8):
                    nc.vector.bn_stats(out=st2[:, i, :],
                                       in_=h3[:, i * 512:(i + 1) * 512])
                mv2 = small.tile([128, 2], F32)
                nc.vector.bn_aggr(out=mv2, in_=st2)
                sums2 = small.tile([128, 2], F32)
                tmp2 = small.tile([128, 1], F32)
                nc.vector.tensor_scalar_mul(out=sums2[:, 0:1], in0=mv2[:, 0:1],
                                            scalar1=4096.0)
                nc.vector.tensor_mul(out=tmp2, in0=mv2[:, 0:1], in1=mv2[:, 0:1])
                nc.vector.tensor_add(out=tmp2, in0=tmp2, in1=mv2[:, 1:2])
                nc.vector.tensor_scalar_mul(out=sums2[:, 1:2], in0=tmp2,
                                            scalar1=4096.0)
                bn2_in = dram.tile([128, 2], F32)
                bn2_out = dram.tile([128, 2], F32)
                nc.gpsimd.dma_start(out=bn2_in, in_=sums2)
                nc.gpsimd.collective_compute(
                    "AllReduce", mybir.AluOpType.add,
                    replica_groups=[list(range(N_CORES))],
                    ins=[bn2_in.opt()], outs=[bn2_out.opt()])
                red2 = small.tile([128, 2], F32)
                nc.gpsimd.dma_start(out=red2, in_=bn2_out)
                s2t2 = small.tile([128, 2], F32)
                _bn_scale_bias(nc, s2t2, red2, w32, BN2_OFF, 32768.0,
                               eps_t, small, rows=128)

            # ---------------- main loop ----------------
            with (
                tc.tile_pool(name="xp", bufs=XP_BUFS) as xp,
                tc.tile_pool(name="dp", bufs=8) as dp,
                tc.tile_pool(name="stp", bufs=4) as stp,
            ):
                def new_x_tile(name):
                    # pad columns (0, 33) of every xp slot were zeroed once
                    # below; applies only write the interior, so they persist.
                    return xp.tile([128, 32 * 34], F32R, tag="X", name=name)

                # one-time zeroing of the pad columns of all X slots: the
                # dummies are simultaneously live (kept alive by the reads
                # below), so by pigeonhole they cover all slots.
                _dummies = []
                for i in range(XP_BUFS):
                    zt = xp.tile([128, 32 * 34], F32R, tag="X", name=f"xz{i}")
                    for xc in (0, 33):
                        pads = bass.AP(tensor=zt.tensor, offset=zt.offset + xc,
                                       ap=[zt.ap[0], [34, 32]])
                        nc.sync.dma_start(out=pads, in_=wcols(Z_OFF, 32))
                    _dummies.append(zt)
                _pad_scratch = small.tile([128, 1], F32)
                for zt in _dummies:
                    nc.scalar.copy(out=_pad_scratch,
                                   in_=zt.bitcast(F32)[:, 0:1])

                Xcur = {}
                for s in range(BPC):
                    xt = new_x_tile(f"X1_0_{s}")
                    xtr = xt.rearrange("p (y x) -> p y x", y=32, x=34)
                    h3r = h3.rearrange("p (s y x) -> p s y x", s=4, y=32, x=32)
                    nc.scalar.activation(out=xtr[:, :, 1:33],
                                         in_=h3r[:, s, :, :],
                                         func=AF.Relu,
                                         scale=s2t2[:, 0:1], bias=s2t2[:, 1:2])
                    Xcur[(0, s)] = xt

                pooled_in = small.tile([128, 2, 4, 4], F32)

                for L in range(1, LAYERS + 1):
                    gs_in = sorted({g for (g, _s) in Xcur})
                    mgs = [2, 3] if L == LAYERS else [0, 1, 2, 3]
                    Xnext = {}
                    Dcur = {}
                    for s in range(BPC):
                        # depthwise conv for each live group
                        for g in gs_in:
                            Xr = Xcur[(g, s)].rearrange("p (y x) -> p y x",
                                                        y=32, x=34)
                            pdw = psA.tile([128, 1024], F32, tag="a",
                                           name=f"pdw{L}_{s}_{g}")
                            pdwr = pdw.rearrange("p (h y x) -> p h y x",
                                                 h=2, y=16, x=32)
                            for h in range(2):
                                for ti, (dy, dx) in enumerate(TAPS):
                                    oy0, oy1 = _clip(h * 16, h * 16 + 16,
                                                     max(0, 1 - dy), 33 - dy)
                                    if oy0 >= oy1:
                                        continue
                                    rhs = Xr[:, oy0 + dy - 1:oy1 + dy - 1,
                                             dx:dx + 32]
                                    outp = pdwr[:, h, oy0 - h * 16:oy1 - h * 16,
                                                :]
                                    t = TAPS.index((dy, dx))
                                    nc.tensor.matmul(
                                        outp,
                                        wcols(WDW_OFF + (g * 9 + t) * 128, 128),
                                        rhs,
                                        start=(ti == 0),
                                        stop=(ti == len(TAPS) - 1))
                            dD = dp.tile([128, 1024], F32R, tag="D",
                                         name=f"D{L}_{s}_{g}")
                            nc.scalar.copy(out=dD, in_=pdw)
                            Dcur[g] = dD
                        # channel mix + instnorm per output group
                        for mg in mgs:
                            pm = psB.tile([128, 1024], F32, tag="b",
                                          name=f"pm{L}_{s}_{mg}")
                            for h in range(2):
                                for ki, kg in enumerate(gs_in):
                                    nc.tensor.matmul(
                                        pm[:, h * 512:h * 512 + 512],
                                        wcols(WMIX_OFF + kg * 512 + mg * 128,
                                              128),
                                        Dcur[kg][:, h * 512:h * 512 + 512],
                                        start=(ki == 0),
                                        stop=(ki == len(gs_in) - 1))
                            st = stp.tile([128, 2, 6], F32, tag="st")
                            nc.vector.bn_stats(out=st[:, 0, :],
                                               in_=pm[:, 0:512])
                            nc.vector.bn_stats(out=st[:, 1, :],
                                               in_=pm[:, 512:1024])
                            mv = stp.tile([128, 2], F32, tag="mv")
                            nc.vector.bn_aggr(out=mv, in_=st)
                            sc = stp.tile([128, 1], F32, tag="sc")
                            tt = stp.tile([128, 1], F32, tag="tt")
                            nc.scalar.activation(out=sc, in_=mv[:, 1:2],
                                                 func=AF.Sqrt, bias=eps_t)
                            nc.vector.reciprocal(out=sc, in_=sc)
                            nc.vector.tensor_scalar_mul(
                                out=sc, in0=sc,
                                scalar1=w32[:, NGB_OFF + 2 * mg:
                                            NGB_OFF + 2 * mg + 1])
                            nc.vector.tensor_mul(out=tt, in0=mv[:, 0:1], in1=sc)
                            nc.vector.tensor_scalar(
                                out=tt, in0=tt, scalar1=-1.0,
                                scalar2=w32[:, NGB_OFF + 2 * mg + 1:
                                            NGB_OFF + 2 * mg + 2],
                                op0=mybir.AluOpType.mult,
                                op1=mybir.AluOpType.add)
                            if L < LAYERS:
                                xt = new_x_tile(f"X{L + 1}_{mg}_{s}")
                                Xnext[(mg, s)] = xt
                                xtr = xt.rearrange("p (y x) -> p y x",
                                                   y=32, x=34)
                                pmr2 = pm.rearrange("p (h y x) -> p h y x",
                                                    h=2, y=16, x=32)
                                for h in range(2):
                                    nc.scalar.activation(
                                        out=xtr[:, h * 16:h * 16 + 16, 1:33],
                                        in_=pmr2[:, h, :, :],
                                        func=AF.Relu, scale=sc, bias=tt)
                            else:
                                pmr = pm.rearrange("p (h y x) -> p h y x",
                                                   h=2, y=16, x=32)
                                nc.scalar.activation(
                                    out=pooled_in[:, mg - 2, s, :],
                                    in_=pmr[:, 0, HALF - 1:HALF + 1,
                                            HALF - 1:HALF + 1],
                                    func=AF.Identity, scale=sc, bias=tt)
                    Xcur = Xnext

                # ---------------- readout ----------------
                tadd = small.tile([128, 2, 4], F32)
                tadd2 = small.tile([128, 2, 4], F32)
                pooled = small.tile([128, 2, 4], F32R)
                nc.vector.tensor_add(out=tadd, in0=pooled_in[:, :, :, 0],
                                     in1=pooled_in[:, :, :, 1])
                nc.vector.tensor_add(out=tadd2, in0=pooled_in[:, :, :, 2],
                                     in1=pooled_in[:, :, :, 3])
                nc.vector.tensor_add(out=pooled, in0=tadd, in1=tadd2)
                y_sb = small.tile([128, 4, 8], F32)
                for mo in range(8):
                    mlen = 128 if mo < 7 else OUT - 7 * 128
                    pf = psA.tile([128, 1024], F32, tag="a", name=f"pf{mo}")
                    for kgi in range(2):
                        nc.tensor.matmul(
                            pf[0:mlen, 0:4],
                            w_sb[:, FCW_OFF + kgi * 1000 + mo * 128:
                                 FCW_OFF + kgi * 1000 + mo * 128 + mlen],
                            pooled[:, kgi, :],
                            start=(kgi == 0), stop=(kgi == 1))
                    nc.scalar.activation(
                        out=y_sb[0:mlen, :, mo], in_=pf[0:mlen, 0:4],
                        func=AF.Identity,
                        bias=w32[0:mlen, FCB_OFF + mo:FCB_OFF + mo + 1],
                        scale=1.0)
                for s in range(BPC):
                    dst1 = bass.AP(tensor=y4.tensor, offset=OUT * s,
                                   ap=[[1, 128], [128, 7]])
                    nc.sync.dma_start(out=dst1, in_=y_sb[:, s, 0:7])
                    dst2 = bass.AP(tensor=y4.tensor, offset=OUT * s + 896,
                                   ap=[[1, 104]])
                    nc.sync.dma_start(out=dst2, in_=y_sb[0:104, s, 7])

    nc.finalize()
    return nc


def di_col(dx):
    # column index of conv1 tap dx within w1x block (emission order 1,0,2)
    return {1: 0, 0: 1, 2: 2}[dx]


def _bn_scale_bias(nc, out_st, sums, w32, gb_off, n_tot, eps_t, pool, rows):
    """out_st[:rows, 0] = gamma*rsqrt(var+eps); out_st[:rows, 1] = beta - mu*scale."""
    r = slice(0, rows)
    mu = pool.tile([128, 1], F32, name=f"mu{gb_off}")
    ex2 = pool.tile([128, 1], F32, name=f"ex2{gb_off}")
    var = pool.tile([128, 1], F32, name=f"var{gb_off}")
    nc.vector.tensor_scalar_mul(out=mu[r], in0=sums[r, 0:1], scalar1=1.0 / n_tot)
    nc.vector.tensor_scalar_mul(out=ex2[r], in0=sums[r, 1:2], scalar1=1.0 / n_tot)
    nc.vector.tensor_mul(out=var[r], in0=mu[r], in1=mu[r])
    nc.vector.tensor_sub(out=var[r], in0=ex2[r], in1=var[r])
    nc.scalar.activation(out=var[r], in_=var[r], func=AF.Sqrt, bias=eps_t[r])
    nc.vector.reciprocal(out=var[r], in_=var[r])
    nc.vector.tensor_scalar_mul(out=out_st[r, 0:1], in0=var[r],
                                scalar1=w32[r, gb_off:gb_off + 1])
    nc.vector.tensor_mul(out=mu[r], in0=mu[r], in1=out_st[r, 0:1])
    nc.vector.tensor_scalar(out=out_st[r, 1:2], in0=mu[r], scalar1=-1.0,
                            scalar2=w32[r, gb_off + 1:gb_off + 2],
                            op0=mybir.AluOpType.mult,
                            op1=mybir.AluOpType.add)


def _pack_weights(ds_w1, ds_w2, ds_w3, conv_w, graph_w, fc_w, fc_b,
                  bn1_g, bn1_b, bn2_g, bn2_b, norm_g, norm_b):
    wts = np.zeros((128, WCOLS), np.float32)
    # pruned graph weight
    k = int((1.0 - PRUNE) * DIM * DIM)
    a = np.abs(graph_w).ravel()
    thresh = np.partition(a, -k)[-k]
    w_eff = np.where(np.abs(graph_w) >= thresh, graph_w, 0.0).astype(np.float32)
    # conv1 taps, paired block-diag:
    # rows 64*q + 27*a + 3*dy + c, cols 64*a + o = w1[o, c, dy, dx]
    for dx in range(3):
        dc = di_col(dx)
        blk = np.zeros((128, 128), np.float32)
        for qq in range(2):
            for aa in range(2):
                for dy in range(3):
                    for c in range(3):
                        blk[64 * qq + 27 * aa + 3 * dy + c,
                            64 * aa:64 * aa + 64] = ds_w1[:, c, dy, dx]
        wts[:, W1X_OFF + dc * 128:W1X_OFF + (dc + 1) * 128] = blk
    # conv2 diag-dup taps
    for t, (dy, dx) in enumerate(TAPS):
        blk = np.zeros((128, 128), np.float32)
        d = ds_w2[:, 0, dy, dx]
        for aa in range(2):
            idx = np.arange(64)
            blk[64 * aa + idx, 64 * aa + idx] = d
        wts[:, W2D_OFF + t * 128:W2D_OFF + (t + 1) * 128] = blk
    # conv3: [64a + c, o] = w3[o, c]
    w3 = ds_w3[:, :, 0, 0]  # [128, 64]
    wts[0:64, W3_OFF:W3_OFF + 128] = w3.T
    wts[64:128, W3_OFF:W3_OFF + 128] = w3.T
    # main dw diag taps
    for g in range(4):
        for t, (dy, dx) in enumerate(TAPS):
            blk = np.zeros((128, 128), np.float32)
            idx = np.arange(128)
            blk[idx, idx] = conv_w[g * 128:(g + 1) * 128, 0, dy, dx]
            off = WDW_OFF + (g * 9 + t) * 128
            wts[:, off:off + 128] = blk
    # mix: [p, kg*512 + mg*128 + j] = w_eff[mg*128 + j, kg*128 + p]
    weT = w_eff.T  # [in, out]
    for kg in range(4):
        wts[:, WMIX_OFF + kg * 512:WMIX_OFF + (kg + 1) * 512] = \
            weT[kg * 128:(kg + 1) * 128, :]
    # fc: [p, kg*1000 + m] = 0.25 * fc_w[m, kg*128 + p]
    for kg in range(2):
        wts[:, FCW_OFF + kg * 1000:FCW_OFF + (kg + 1) * 1000] = \
            0.25 * fc_w[:, kg * 128:(kg + 1) * 128].T
    # fc bias [p, mo]
    fcb = np.zeros((128, 8), np.float32)
    fb = np.zeros(1024, np.float32)
    fb[:OUT] = fc_b
    fcb[:, :] = fb.reshape(8, 128).T
    wts[:, FCB_OFF:FCB_OFF + 8] = fcb
    # bn gammas/betas
    wts[0:64, BN1_OFF] = bn1_g
    wts[64:128, BN1_OFF] = bn1_g
    wts[0:64, BN1_OFF + 1] = bn1_b
    wts[64:128, BN1_OFF + 1] = bn1_b
    wts[:, BN2_OFF] = bn2_g
    wts[:, BN2_OFF + 1] = bn2_b
    for g in range(4):
        wts[:, NGB_OFF + 2 * g] = norm_g[g * 128:(g + 1) * 128]
        wts[:, NGB_OFF + 2 * g + 1] = norm_b[g * 128:(g + 1) * 128]
    return wts


_nc_cache = None
last_results = None


def kernel(**inputs):
    global _nc_cache, last_results
    inputs = {k: np.asarray(v, np.float32) for k, v in inputs.items()}
    wts = _pack_weights(
        inputs["ds_w1"], inputs["ds_w2"], inputs["ds_w3"], inputs["conv_w"],
        inputs["graph_w"], inputs["fc_w"], inputs["fc_b"],
        inputs["bn1_g"], inputs["bn1_b"], inputs["bn2_g"], inputs["bn2_b"],
        inputs["norm_g"], inputs["norm_b"])
    x = inputs["x"]
    if _nc_cache is None:
        _nc_cache = build_nc()
    nc = _nc_cache
    in_maps = [{"x4": np.ascontiguousarray(x[c * BPC:(c + 1) * BPC]),
                "wts": wts} for c in range(N_CORES)]
    res = run_bass_kernel_spmd(nc, in_maps, core_ids=list(range(N_CORES)))
    last_results = res
    return np.concatenate([res.results[c]["y4"] for c in range(N_CORES)], axis=0)


# revision 15
# speedup vs baseline: 1323.6849x; 1.0032x over previous
"""Trainium2 Bass kernel for nn_DiscreteTimeNeuralGraph.

Strategy (8 NeuronCores, batch-parallel):
  - Shard the batch of 32 across 8 cores (4 samples each). All weights
    replicated via one mega-weight DMA.
  - Downsample path on-device; BatchNorm batch stats via per-core partial
    sums + one tiny AllReduce each (BN biases cancel through the BNs and
    are dropped).
  - 8 graph layers: depthwise 3x3 conv as 9 rect-clipped diagonal matmuls
    on PE accumulating in PSUM; channel mix (pruned 512x512 weight, dense)
    as blocked matmuls; instance-norm stats on VectorE (bn_stats on PSUM);
    instnorm+ReLU fused into one ScalarE activation reading PSUM and
    writing the next layer's activations.
  - Matmul inputs are float32r (~tf32 precision, 1 col/cycle on PE).
  - Readout: center 2x2 mean (folded into fc weights) + fc matmul.

Top-k threshold for the pruned graph weight is computed on host
(np.partition) -- it is weight preprocessing of a replicated input.
"""
import numpy as np

import concourse.bass as bass
import concourse.tile as tile
from concourse import bacc, mybir
from concourse.bass_utils import run_bass_kernel_spmd

F32 = mybir.dt.float32
F32R = mybir.dt.float32r
AF = mybir.ActivationFunctionType

N_CORES = 8
B = 32
BPC = B // N_CORES          # 4 samples per core
DIM = 512
DS = 128
FEAT = 256
LAYERS = 8
IMG = 128
OUT = 1000
EPS = 1e-5
HALF = IMG // 4 // 2 - 1    # 15
PRUNE = 0.9

# mega-weight column layout (f32r, [128, WCOLS])
W1X_OFF = 0                  # 3 dx-taps x [128,128] for conv1
W2D_OFF = W1X_OFF + 3 * 128  # 9 taps x [128,128] diag-dup for conv2
W3_OFF = W2D_OFF + 9 * 128   # [128,128] conv3 (w3 stacked twice on K)
WDW_OFF = W3_OFF + 128       # 4 groups x 9 taps x [128,128] diag for main dw
WMIX_OFF = WDW_OFF + 36 * 128  # 4 kg x [128, 512] = w_eff.T blocks
FCW_OFF = WMIX_OFF + 4 * 512   # 2 kg x [128, 1000] fc lhsT (x0.25 pooled)
FCB_OFF = FCW_OFF + 2 * 1000   # [128, 8] fc bias chunks
BN1_OFF = FCB_OFF + 8          # [128, 2] bn1 gamma/beta (dup across halves)
BN2_OFF = BN1_OFF + 2          # [128, 2]
NGB_OFF = BN2_OFF + 2          # [128, 8] instnorm gamma/beta per group
Z_OFF = NGB_OFF + 8            # [128, 256] zeros (pad fills)
WCOLS = Z_OFF + 256

XP_BUFS = 20

# tap order: full-coverage tap first (start=True zeroes the psum region)
TAPS = [(1, 1), (0, 0), (0, 1), (0, 2), (1, 0), (1, 2), (2, 0), (2, 1), (2, 2)]


def _clip(lo, hi, lo2, hi2):
    return max(lo, lo2), min(hi, hi2)


def build_nc():
    nc = bacc.Bacc(num_devices=N_CORES)
    x4 = nc.dram_tensor("x4", [BPC, 3, IMG, IMG], F32R, kind="ExternalInput").ap()
    wts = nc.dram_tensor("wts", [128, WCOLS], F32R, kind="ExternalInput").ap()
    y4 = nc.dram_tensor("y4", [BPC, OUT], F32, kind="ExternalOutput").ap()

    with tile.TileContext(nc) as tc:
        with (
            tc.tile_pool(name="wp", bufs=1) as wp,
            tc.tile_pool(name="small", bufs=1) as small,
            tc.tile_pool(name="psA", bufs=2, space="PSUM") as psA,
            tc.tile_pool(name="psB", bufs=3, space="PSUM") as psB,
            tc.tile_pool(name="dram", bufs=1, space="DRAM") as dram,
        ):
            w_sb = wp.tile([128, WCOLS], F32R)
            nc.sync.dma_start(out=w_sb, in_=wts)
            w32 = w_sb.bitcast(F32)

            def wcols(off, n):
                return w_sb[:, off:off + n]

            def zsrc(d1, d2):
                return bass.AP(tensor=w_sb.tensor, offset=w_sb.offset + Z_OFF,
                               ap=[w_sb.ap[0], [d2, d1], [1, d2]])

            eps_t = small.tile([128, 1], F32)
            nc.vector.memset(eps_t, EPS)

            # ---------------- downsample ----------------
            with tc.tile_pool(name="ds1", bufs=1) as ds1:
                # im2col9: partition p = 32*s + 3*dy + c ; free = (oy 64, ix' 130)
                # ix' = ix + 1 (x padded by 1 on both sides)
                im9 = ds1.tile([128, 64 * 130], F32R)
                im9r = im9.rearrange("p (y x) -> p y x", y=64, x=130)
                # zero the x pads (cols 0 and 129) in one DMA
                for xc in (0, 129):
                    im9_pads = bass.AP(tensor=im9.tensor,
                                       offset=im9.offset + xc,
                                       ap=[im9.ap[0], [130, 64]])
                    nc.sync.dma_start(out=im9_pads, in_=wcols(Z_OFF, 64))
                # x rows: iy = 2*oy + dy - 1
                # partition base: sample s -> 64*(s%2) + 27*(s//2)
                x4r = x4.rearrange("s c (y2 two) x -> s c y2 two x", two=2)
                for s in range(BPC):
                    for dy in range(3):
                        p0 = 64 * (s % 2) + 27 * (s // 2) + 3 * dy
                        if dy == 0:
                            # oy in [1,64): iy = 2*(oy-1)+1
                            nc.sync.dma_start(
                                out=im9r[p0:p0 + 3, 1:64, 1:129],
                                in_=x4r[s, :, 0:63, 1, :])
                            # row oy=0 is out of range: zero it
                            nc.sync.dma_start(out=im9r[p0:p0 + 3, 0:1, :],
                                              in_=wcols(Z_OFF, 130)[p0:p0 + 3])
                        elif dy == 1:
                            nc.sync.dma_start(
                                out=im9r[p0:p0 + 3, :, 1:129],
                                in_=x4r[s, :, :, 0, :])
                        else:
                            nc.sync.dma_start(
                                out=im9r[p0:p0 + 3, :, 1:129],
                                in_=x4r[s, :, :, 1, :])

                # conv1: out h1 [128 = 64*(s//2)+ch, (s%2, oy 64, ox 64)]
                h1 = ds1.tile([128, 8192], F32)
                h1r = h1.rearrange("p (sh y x) -> p sh y x", sh=2, y=64, x=64)
                # im2col x-read: ix' = 2*ox + dx (x2 = ox + dx//2, tx = dx%2)
                # paired matmul: K=54 block-diag covers samples (q, q+2):
                # out partitions 0-63 <- sample q, 64-127 <- sample q+2.
                im9x = im9.rearrange("p (y x2 two) -> p y x2 two", x2=65, two=2)
                for q in range(2):
                    for yb in range(4):           # 16-oy blocks
                        for h in range(2):
                            pc1 = psA.tile([128, 512], F32, tag="a",
                                           name="pc1")
                            pc1r = pc1.rearrange("p (y x) -> p y x", y=8, x=64)
                            oy0 = yb * 16 + h * 8
                            for di, dx in enumerate([1, 0, 2]):
                                rhs = im9x[64 * q:64 * q + 54, oy0:oy0 + 8,
                                           dx // 2:dx // 2 + 64, dx % 2]
                                lhsT = w_sb[64 * q:64 * q + 54,
                                            W1X_OFF + di_col(dx) * 128:
                                            W1X_OFF + di_col(dx) * 128 + 128]
                                nc.tensor.matmul(pc1r, lhsT, rhs,
                                                 start=(di == 0), stop=(di == 2),
                                                 tile_position=(64 * q, 0))
                            if (q + yb + h) % 2 == 0:
                                nc.scalar.copy(
                                    out=h1r[:, q, oy0:oy0 + 8, :], in_=pc1)
                            else:
                                nc.vector.tensor_copy(
                                    out=h1r[:, q, oy0:oy0 + 8, :], in_=pc1)

                # BN1 partial stats
                st1 = small.tile([128, 16, 6], F32)
                for i in range(16):
                    nc.vector.bn_stats(out=st1[:, i, :],
                                       in_=h1[:, i * 512:(i + 1) * 512])
                mv1 = small.tile([128, 2], F32)
                nc.vector.bn_aggr(out=mv1, in_=st1)
                sums1 = small.tile([128, 2], F32)
                tmp1 = small.tile([128, 1], F32)
                nc.vector.tensor_scalar_mul(out=sums1[:, 0:1], in0=mv1[:, 0:1],
                                            scalar1=8192.0)
                nc.vector.tensor_mul(out=tmp1, in0=mv1[:, 0:1], in1=mv1[:, 0:1])
                nc.vector.tensor_add(out=tmp1, in0=tmp1, in1=mv1[:, 1:2])
                nc.vector.tensor_scalar_mul(out=sums1[:, 1:2], in0=tmp1,
                                            scalar1=8192.0)
                bn1_in = dram.tile([128, 2], F32)
                bn1_out = dram.tile([128, 2], F32)
                nc.gpsimd.dma_start(out=bn1_in, in_=sums1)
                nc.gpsimd.collective_compute(
                    "AllReduce", mybir.AluOpType.add,
                    replica_groups=[list(range(N_CORES))],
                    ins=[bn1_in.opt()], outs=[bn1_out.opt()])
                red1 = small.tile([128, 2], F32)
                nc.gpsimd.dma_start(out=red1, in_=bn1_out)
                comb1 = small.tile([128, 2], F32)
                nc.gpsimd.dma_start(out=comb1[0:64, :], in_=red1[0:64, :])
                nc.gpsimd.dma_start(out=comb1[0:64, :], in_=red1[64:128, :],
                                    accum_op=mybir.AluOpType.add)
                # scale/bias on rows 0:64, then duplicate
                s1t1 = small.tile([128, 2], F32)
                _bn_scale_bias(nc, s1t1, comb1, w32, BN1_OFF, 131072.0,
                               eps_t, small, rows=64)
                nc.gpsimd.dma_start(out=s1t1[64:128, :], in_=s1t1[0:64, :])

                # apply BN1 + relu -> h1n (f32r), x padded to 66 (ix' = ix+1)
                h1n = ds1.tile([128, 2 * 64 * 66], F32R)
                h1nr3 = h1n.rearrange("p (sh y x) -> p sh y x",
                                      sh=2, y=64, x=66)
                for sh in range(2):
                    for xc in (0, 65):
                        h1n_pads = bass.AP(tensor=h1n.tensor,
                                           offset=h1n.offset + 4224 * sh + xc,
                                           ap=[h1n.ap[0], [66, 64]])
                        nc.sync.dma_start(out=h1n_pads, in_=wcols(Z_OFF, 64))
                h1r4 = h1.rearrange("p (sh y x) -> p sh y x", sh=2, y=64, x=64)
                for sh in range(2):
                    nc.scalar.activation(out=h1nr3[:, sh, :, 1:65],
                                         in_=h1r4[:, sh, :, :], func=AF.Relu,
                                         scale=s1t1[:, 0:1], bias=s1t1[:, 1:2])

                # conv2: depthwise 3x3 stride 2 -> d2 [128, (sh, 32, 32)]
                # row iy = 2*oy + dy - 1 (unpadded), col ix' = 2*ox + dx (padded)
                h1nr = h1n.rearrange(
                    "p (sh y2 ty x2 tx) -> p sh y2 ty x2 tx",
                    sh=2, y2=32, ty=2, x2=33, tx=2)
                d2 = ds1.tile([128, 2048], F32R)
                for sh in range(2):
                    pd2 = psB.tile([128, 1024], F32, tag="b", name="pd2")
                    pd2r = pd2.rearrange("p (h y x) -> p h y x", h=2, y=16, x=32)
                    for h in range(2):
                        for ti, (dy, dx) in enumerate(TAPS):
                            oy0, oy1 = _clip(h * 16, h * 16 + 16,
                                             1 if dy == 0 else 0, 32)
                            if dy == 1:
                                ys, par = oy0, 0
                            elif dy == 0:
                                ys, par = oy0 - 1, 1
                            else:
                                ys, par = oy0, 1
                            rhs = h1nr[:, sh, ys:ys + (oy1 - oy0), par,
                                       dx // 2:dx // 2 + 32, dx % 2]
                            outp = pd2r[:, h, oy0 - h * 16:oy1 - h * 16, :]
                            t = TAPS.index((dy, dx))
                            nc.tensor.matmul(
                                outp, wcols(W2D_OFF + t * 128, 128), rhs,
                                start=(ti == 0), stop=(ti == len(TAPS) - 1))
                    nc.scalar.copy(out=d2[:, sh * 1024:(sh + 1) * 1024], in_=pd2)

                # conv3: 1x1, 64 -> 128 ; h3 [128=outc, (s, 1024px)]
                h3 = small.tile([128, 4096], F32)
                for a in range(2):
                    for nb in range(4):
                        pc3 = psA.tile([128, 512], F32, tag="a",
                                       name=f"pc3_{a}_{nb}")
                        nc.tensor.matmul(
                            pc3,
                            w_sb[64 * a:64 * a + 64, W3_OFF:W3_OFF + 128],
                            d2[64 * a:64 * a + 64, nb * 512:(nb + 1) * 512],
                            start=True, stop=True)
                        s_full = 2 * a + nb // 2
                        dst = h3[:, s_full * 1024 + (nb % 2) * 512:
                                 s_full * 1024 + (nb % 2) * 512 + 512]
                        if nb % 2 == 0:
                            nc.scalar.copy(out=dst, in_=pc3)
                        else:
                            nc.vector.tensor_copy(out=dst, in_=pc3)

                # BN2 stats + allreduce
                st2 = small.tile([128, 8, 6], F32)
                for i in range(8):
                    nc.vector.bn_stats(out=st2[:, i, :],
                                       in_=h3[:, i * 512:(i + 1) * 512])
                mv2 = small.tile([128, 2], F32)
                nc.vector.bn_aggr(out=mv2, in_=st2)
                sums2 = small.tile([128, 2], F32)
                tmp2 = small.tile([128, 1], F32)
                nc.vector.tensor_scalar_mul(out=sums2[:, 0:1], in0=mv2[:, 0:1],
                                            scalar1=4096.0)
                nc.vector.tensor_mul(out=tmp2, in0=mv2[:, 0:1], in1=mv2[:, 0:1])
                nc.vector.tensor_add(out=tmp2, in0=tmp2, in1=mv2[:, 1:2])
                nc.vector.tensor_scalar_mul(out=sums2[:, 1:2], in0=tmp2,
                                            scalar1=4096.0)
                bn2_in = dram.tile([128, 2], F32)
                bn2_out = dram.tile([128, 2], F32)
                nc.gpsimd.dma_start(out=bn2_in, in_=sums2)
                nc.gpsimd.collective_compute(
                    "AllReduce", mybir.AluOpType.add,
                    replica_groups=[list(range(N_CORES))],
                    ins=[bn2_in.opt()], outs=[bn2_out.opt()])
                red2 = small.tile([128, 2], F32)
                nc.gpsimd.dma_start(out=red2, in_=bn2_out)
                s2t2 = small.tile([128, 2], F32)
                _bn_scale_bias(nc, s2t2, red2, w32, BN2_OFF, 32768.0,
                               eps_t, small, rows=128)

            # ---------------- main loop ----------------
            with (
                tc.tile_pool(name="xp", bufs=XP_BUFS) as xp,
                tc.tile_pool(name="dp", bufs=8) as dp,
                tc.tile_pool(name="stp", bufs=4) as stp,
            ):
                def new_x_tile(name):
                    # pad columns (0, 33) of every xp slot were zeroed once
                    # below; applies only write the interior, so they persist.
                    return xp.tile([128, 32 * 34], F32R, tag="X", name=name)

                # one-time zeroing of the pad columns of all X slots: the
                # dummies are simultaneously live (kept alive by the reads
                # below), so by pigeonhole they cover all slots.
                _dummies = []
                for i in range(XP_BUFS):
                    zt = xp.tile([128, 32 * 34], F32R, tag="X", name=f"xz{i}")
                    for xc in (0, 33):
                        pads = bass.AP(tensor=zt.tensor, offset=zt.offset + xc,
                                       ap=[zt.ap[0], [34, 32]])
                        nc.sync.dma_start(out=pads, in_=wcols(Z_OFF, 32))
                    _dummies.append(zt)
                _pad_scratch = small.tile([128, 1], F32)
                for zt in _dummies:
                    nc.scalar.copy(out=_pad_scratch,
                                   in_=zt.bitcast(F32)[:, 0:1])

                Xcur = {}
                for s in range(BPC):
                    xt = new_x_tile(f"X1_0_{s}")
                    xtr = xt.rearrange("p (y x) -> p y x", y=32, x=34)
                    h3r = h3.rearrange("p (s y x) -> p s y x", s=4, y=32, x=32)
                    nc.scalar.activation(out=xtr[:, :, 1:33],
                                         in_=h3r[:, s, :, :],
                                         func=AF.Relu,
                                         scale=s2t2[:, 0:1], bias=s2t2[:, 1:2])
                    Xcur[(0, s)] = xt

                pooled_in = small.tile([128, 2, 4, 4], F32)

                for L in range(1, LAYERS + 1):
                    gs_in = sorted({g for (g, _s) in Xcur})
                    mgs = [2, 3] if L == LAYERS else [0, 1, 2, 3]
                    Xnext = {}
                    Dcur = {}
                    for s in range(BPC):
                        # depthwise conv for each live group
                        for g in gs_in:
                            Xr = Xcur[(g, s)].rearrange("p (y x) -> p y x",
                                                        y=32, x=34)
                            dD = dp.tile([128, 1024], F32R, tag="D",
                                         name=f"D{L}_{s}_{g}")
                            for h in range(2):
                                pdw = psA.tile([128, 512], F32, tag="a",
                                               name=f"pdw{L}_{s}_{g}_{h}")
                                pdwr = pdw.rearrange("p (y x) -> p y x",
                                                     y=16, x=32)
                                for ti, (dy, dx) in enumerate(TAPS):
                                    oy0, oy1 = _clip(h * 16, h * 16 + 16,
                                                     max(0, 1 - dy), 33 - dy)
                                    if oy0 >= oy1:
                                        continue
                                    rhs = Xr[:, oy0 + dy - 1:oy1 + dy - 1,
                                             dx:dx + 32]
                                    outp = pdwr[:, oy0 - h * 16:oy1 - h * 16, :]
                                    t = TAPS.index((dy, dx))
                                    nc.tensor.matmul(
                                        outp,
                                        wcols(WDW_OFF + (g * 9 + t) * 128, 128),
                                        rhs,
                                        start=(ti == 0),
                                        stop=(ti == len(TAPS) - 1))
                                nc.scalar.copy(
                                    out=dD[:, h * 512:h * 512 + 512], in_=pdw)
                            Dcur[g] = dD
                        # channel mix + instnorm per output group
                        for mg in mgs:
                            pm = psB.tile([128, 1024], F32, tag="b",
                                          name=f"pm{L}_{s}_{mg}")
                            for h in range(2):
                                for ki, kg in enumerate(gs_in):
                                    nc.tensor.matmul(
                                        pm[:, h * 512:h * 512 + 512],
                                        wcols(WMIX_OFF + kg * 512 + mg * 128,
                                              128),
                                        Dcur[kg][:, h * 512:h * 512 + 512],
                                        start=(ki == 0),
                                        stop=(ki == len(gs_in) - 1))
                            st = stp.tile([128, 2, 6], F32, tag="st")
                            nc.vector.bn_stats(out=st[:, 0, :],
                                               in_=pm[:, 0:512])
                            nc.vector.bn_stats(out=st[:, 1, :],
                                               in_=pm[:, 512:1024])
                            mv = stp.tile([128, 2], F32, tag="mv")
                            nc.vector.bn_aggr(out=mv, in_=st)
                            sc = stp.tile([128, 1], F32, tag="sc")
                            tt = stp.tile([128, 1], F32, tag="tt")
                            nc.scalar.activation(out=sc, in_=mv[:, 1:2],
                                                 func=AF.Sqrt, bias=eps_t)
                            nc.vector.reciprocal(out=sc, in_=sc)
                            nc.vector.tensor_scalar_mul(
                                out=sc, in0=sc,
                                scalar1=w32[:, NGB_OFF + 2 * mg:
                                            NGB_OFF + 2 * mg + 1])
                            nc.vector.tensor_mul(out=tt, in0=mv[:, 0:1], in1=sc)
                            nc.vector.tensor_scalar(
                                out=tt, in0=tt, scalar1=-1.0,
                                scalar2=w32[:, NGB_OFF + 2 * mg + 1:
                                            NGB_OFF + 2 * mg + 2],
                                op0=mybir.AluOpType.mult,
                                op1=mybir.AluOpType.add)
                            if L < LAYERS:
                                xt = new_x_tile(f"X{L + 1}_{mg}_{s}")
                                Xnext[(mg, s)] = xt
                                xtr = xt.rearrange("p (y x) -> p y x",
                                                   y=32, x=34)
                                pmr2 = pm.rearrange("p (y x) -> p y x",
                                                    y=32, x=32)
                                nc.scalar.activation(
                                    out=xtr[:, :, 1:33], in_=pmr2,
                                    func=AF.Relu, scale=sc, bias=tt)
                            else:
                                pmr = pm.rearrange("p (h y x) -> p h y x",
                                                   h=2, y=16, x=32)
                                nc.scalar.activation(
                                    out=pooled_in[:, mg - 2, s, :],
                                    in_=pmr[:, 0, HALF - 1:HALF + 1,
                                            HALF - 1:HALF + 1],
                                    func=AF.Identity, scale=sc, bias=tt)
                    Xcur = Xnext

                # ---------------- readout ----------------
                tadd = small.tile([128, 2, 4], F32)
                tadd2 = small.tile([128, 2, 4], F32)
                pooled = small.tile([128, 2, 4], F32R)
                nc.vector.tensor_add(out=tadd, in0=pooled_in[:, :, :, 0],
                                     in1=pooled_in[:, :, :, 1])
                nc.vector.tensor_add(out=tadd2, in0=pooled_in[:, :, :, 2],
                                     in1=pooled_in[:, :, :, 3])
                nc.vector.tensor_add(out=pooled, in0=tadd, in1=tadd2)
                y_sb = small.tile([128, 4, 8], F32)
                for mo in range(8):
                    mlen = 128 if mo < 7 else OUT - 7 * 128
                    pf = psA.tile([128, 512], F32, tag="a", name=f"pf{mo}")
                    for kgi in range(2):
                        nc.tensor.matmul(
                            pf[0:mlen, 0:4],
                            w_sb[:, FCW_OFF + kgi * 1000 + mo * 128:
                                 FCW_OFF + kgi * 1000 + mo * 128 + mlen],
                            pooled[:, kgi, :],
                            start=(kgi == 0), stop=(kgi == 1))
                    nc.scalar.activation(
                        out=y_sb[0:mlen, :, mo], in_=pf[0:mlen, 0:4],
                        func=AF.Identity,
                        bias=w32[0:mlen, FCB_OFF + mo:FCB_OFF + mo + 1],
                        scale=1.0)
                for s in range(BPC):
                    dst1 = bass.AP(tensor=y4.tensor, offset=OUT * s,
                                   ap=[[1, 128], [128, 7]])
                    nc.sync.dma_start(out=dst1, in_=y_sb[:, s, 0:7])
                    dst2 = bass.AP(tensor=y4.tensor, offset=OUT * s + 896,
                                   ap=[[1, 104]])
                    nc.sync.dma_start(out=dst2, in_=y_sb[0:104, s, 7])

    nc.finalize()
    return nc


def di_col(dx):
    # column index of conv1 tap dx within w1x block (emission order 1,0,2)
    return {1: 0, 0: 1, 2: 2}[dx]


def _bn_scale_bias(nc, out_st, sums, w32, gb_off, n_tot, eps_t, pool, rows):
    """out_st[:rows, 0] = gamma*rsqrt(var+eps); out_st[:rows, 1] = beta - mu*scale."""
    r = slice(0, rows)
    mu = pool.tile([128, 1], F32, name=f"mu{gb_off}")
    ex2 = pool.tile([128, 1], F32, name=f"ex2{gb_off}")
    var = pool.tile([128, 1], F32, name=f"var{gb_off}")
    nc.vector.tensor_scalar_mul(out=mu[r], in0=sums[r, 0:1], scalar1=1.0 / n_tot)
    nc.vector.tensor_scalar_mul(out=ex2[r], in0=sums[r, 1:2], scalar1=1.0 / n_tot)
    nc.vector.tensor_mul(out=var[r], in0=mu[r], in1=mu[r])
    nc.vector.tensor_sub(out=var[r], in0=ex2[r], in1=var[r])
    nc.scalar.activation(out=var[r], in_=var[r], func=AF.Sqrt, bias=eps_t[r])
    nc.vector.reciprocal(out=var[r], in_=var[r])
    nc.vector.tensor_scalar_mul(out=out_st[r, 0:1], in0=var[r],
                                scalar1=w32[r, gb_off:gb_off + 1])
    nc.vector.tensor_mul(out=mu[r], in0=mu[r], in1=out_st[r, 0:1])
    nc.vector.tensor_scalar(out=out_st[r, 1:2], in0=mu[r], scalar1=-1.0,
                            scalar2=w32[r, gb_off + 1:gb_off + 2],
                            op0=mybir.AluOpType.mult,
                            op1=mybir.AluOpType.add)


def _pack_weights(ds_w1, ds_w2, ds_w3, conv_w, graph_w, fc_w, fc_b,
                  bn1_g, bn1_b, bn2_g, bn2_b, norm_g, norm_b):
    wts = np.zeros((128, WCOLS), np.float32)
    # pruned graph weight
    k = int((1.0 - PRUNE) * DIM * DIM)
    a = np.abs(graph_w).ravel()
    thresh = np.partition(a, -k)[-k]
    w_eff = np.where(np.abs(graph_w) >= thresh, graph_w, 0.0).astype(np.float32)
    # conv1 taps, paired block-diag:
    # rows 64*q + 27*a + 3*dy + c, cols 64*a + o = w1[o, c, dy, dx]
    for dx in range(3):
        dc = di_col(dx)
        blk = np.zeros((128, 128), np.float32)
        for qq in range(2):
            for aa in range(2):
                for dy in range(3):
                    for c in range(3):
                        blk[64 * qq + 27 * aa + 3 * dy + c,
                            64 * aa:64 * aa + 64] = ds_w1[:, c, dy, dx]
        wts[:, W1X_OFF + dc * 128:W1X_OFF + (dc + 1) * 128] = blk
    # conv2 diag-dup taps
    for t, (dy, dx) in enumerate(TAPS):
        blk = np.zeros((128, 128), np.float32)
        d = ds_w2[:, 0, dy, dx]
        for aa in range(2):
            idx = np.arange(64)
            blk[64 * aa + idx, 64 * aa + idx] = d
        wts[:, W2D_OFF + t * 128:W2D_OFF + (t + 1) * 128] = blk
    # conv3: [64a + c, o] = w3[o, c]
    w3 = ds_w3[:, :, 0, 0]  # [128, 64]
    wts[0:64, W3_OFF:W3_OFF + 128] = w3.T
    wts[64:128, W3_OFF:W3_OFF + 128] = w3.T
    # main dw diag taps
    for g in range(4):
        for t, (dy, dx) in enumerate(TAPS):
            blk = np.zeros((128, 128), np.float32)
            idx = np.arange(128)
            blk[idx, idx] = conv_w[g * 128:(g + 1) * 128, 0, dy, dx]
            off = WDW_OFF + (g * 9 + t) * 128
            wts[:, off:off + 128] = blk
    # mix: [p, kg*512 + mg*128 + j] = w_eff[mg*128 + j, kg*128 + p]
    weT = w_eff.T  # [in, out]
    for kg in range(4):
        wts[:, WMIX_OFF + kg * 512:WMIX_OFF + (kg + 1) * 512] = \
            weT[kg * 128:(kg + 1) * 128, :]
    # fc: [p, kg*1000 + m] = 0.25 * fc_w[m, kg*128 + p]
    for kg in range(2):
        wts[:, FCW_OFF + kg * 1000:FCW_OFF + (kg + 1) * 1000] = \
            0.25 * fc_w[:, kg * 128:(kg + 1) * 128].T
    # fc bias [p, mo]
    fcb = np.zeros((128, 8), np.float32)
    fb = np.zeros(1024, np.float32)
    fb[:OUT] = fc_b
    fcb[:, :] = fb.reshape(8, 128).T
    wts[:, FCB_OFF:FCB_OFF + 8] = fcb
    # bn gammas/betas
    wts[0:64, BN1_OFF] = bn1_g
    wts[64:128, BN1_OFF] = bn1_g
    wts[0:64, BN1_OFF + 1] = bn1_b
    wts[64:128, BN1_OFF + 1] = bn1_b
    wts[:, BN2_OFF] = bn2_g
    wts[:, BN2_OFF + 1] = bn2_b
    for g in range(4):
        wts[:, NGB_OFF + 2 * g] = norm_g[g * 128:(g + 1) * 128]
        wts[:, NGB_OFF + 2 * g + 1] = norm_b[g * 128:(g + 1) * 128]
    return wts


_nc_cache = None
last_results = None


def kernel(**inputs):
    global _nc_cache, last_results
    inputs = {k: np.asarray(v, np.float32) for k, v in inputs.items()}
    wts = _pack_weights(
        inputs["ds_w1"], inputs["ds_w2"], inputs["ds_w3"], inputs["conv_w"],
        inputs["graph_w"], inputs["fc_w"], inputs["fc_b"],
        inputs["bn1_g"], inputs["bn1_b"], inputs["bn2_g"], inputs["bn2_b"],
        inputs["norm_g"], inputs["norm_b"])
    x = inputs["x"]
    if _nc_cache is None:
        _nc_cache = build_nc()
    nc = _nc_cache
    in_maps = [{"x4": np.ascontiguousarray(x[c * BPC:(c + 1) * BPC]),
                "wts": wts} for c in range(N_CORES)]
    res = run_bass_kernel_spmd(nc, in_maps, core_ids=list(range(N_CORES)))
    last_results = res
    return np.concatenate([res.results[c]["y4"] for c in range(N_CORES)], axis=0)
